# revision 8
# baseline (speedup 1.0000x reference)
"""Embedding-lookup dot product kernel for 8 TRN2 NeuronCores.

out[i] = dot(user_matrix[location[i,0], :], goods_matrix[:, location[i,1]])

Mechanism discovery (HW-verified): the INDIRECT1D SWDGE instruction consumes
exactly ONE offset per partition (128 rows / ~1.1us Pool-serial each -> the
58.9us baseline needs 32 of them), while the Anthropic extended DMA_GATHER
instruction amortizes descriptor generation (~1us fixed + 0.34ns/row, 8192
rows in one instruction) but takes int16 indices, so one instruction only
addresses a 32768-row (16MB) window of the source.

Strategy (model-parallel over table windows, per the sharding hint): pad each
factor matrix to 16 windows x 32768 rows. Arrange the 8 cores in a 4x2 grid:
core (i,j) owns u-windows {4i..4i+3} (64MB) and g-windows {8j..8j+7} (128MB)
of the transposed goods matrix -- a fixed, data-independent sharding. Every
pair (wu, wg) then belongs to exactly one core, and each core's gather work
is 12 dma_gather instructions (4 u + 8 g) instead of 32 INDIRECT1Ds.

Within a core, pairs are bucketed by cell (wu_local, wg_local) (4x8=32
cells), each cell padded to 128 slots so one cell = one SBUF column of the
gathered tile (dma_gather writes slot i to partition i%128, column i//128).
The U tile is filled wu-major, the G tile wg-major; the DVE multiplies the
G tile against a stride-transposed AP view of the U tile (cells align), one
chunk per g-window as it lands, then reduces over K into res. Host maps
res[rank, cell] back to pair order and drops padding slots.
"""

from contextlib import ExitStack

import numpy as np

import concourse.bacc as bacc
import concourse.mybir as mybir
from concourse import bass
from concourse.library_config import mlp
from concourse.bass_utils import run_bass_kernel_spmd

N_CORES = 8
USER_NUM = 500000
GOODS_NUM = 500000
K = 128               # embedding dim
BATCH = 16384
P = 128               # SBUF partitions

WIN = 32768           # int16-addressable rows per dma_gather window
NWIN = 16             # windows per factor matrix (16*32768 = 524288 >= 500000)
UG, GG = 4, 8         # u-windows and g-windows per core (4x2 core grid)
CELLS = UG * GG       # 32 cells per core
CAP = 128             # pair slots per cell (one SBUF column)
NU = GG * CAP         # idxs per u-gather instruction (1024)
NG = UG * CAP         # idxs per g-gather instruction (512)
UCOLS = NU // 16      # idx tile columns per u instruction (64)
GCOLS = NG // 16      # (32)
IDXCOLS = UG * UCOLS + GG * GCOLS   # 512

_CACHE = {}


def build_nc():
    """Build + compile the per-core Bass graph (identical on all 8 cores;
    only the tensor contents differ per core)."""
    f32 = mybir.dt.float32
    i16 = mybir.dt.int16

    # Skip the constructor's all-engine barrier: every cross-engine
    # dependency in this kernel is explicitly sem-ordered, and the const-AP
    # memsets it protects are unused here.
    orig_barrier = bass.Bass.all_engine_barrier
    bass.Bass.all_engine_barrier = lambda self, *, sem_only=False: None
    try:
        nc = bacc.Bacc(
            "TRN2",
            target_bir_lowering=False,
            debug=False,
            num_devices=N_CORES,
            enable_partition_id=False,
            monotonic_sem_count=0,
            num_swdge_queues=4,
        )
    finally:
        bass.Bass.all_engine_barrier = orig_barrier

    # per-core table: this core's 4 u-windows then its 8 g-windows
    tab = nc.dram_tensor("tab", [(UG + GG) * WIN, K], f32, kind="ExternalInput")
    loc = nc.dram_tensor("loc", [P, IDXCOLS], i16, kind="ExternalInput")
    out = nc.dram_tensor("out", [P, CELLS], f32, kind="ExternalOutput")

    with (
        # all SWDGE completions are sem-proven before the store, so skip the
        # expensive gpsimd dge_drain at block end
        nc.Block(no_gpsimd_drain=True) as block,
        nc.sbuf_tensor("idx", [P, IDXCOLS], i16) as idx,
        nc.sbuf_tensor("gatU", [P, UG * GG, K], f32) as gatU,   # wu-major cells
        nc.sbuf_tensor("gatG", [P, GG * UG, K], f32) as gatG,   # wg-major cells
        nc.sbuf_tensor("res", [P, CELLS], f32) as res,
        nc.sbuf_tensor("warm", [P, 1, K], f32) as warm,
        nc.semaphore("io") as io,
        nc.semaphore("wsem") as wsem,
        nc.semaphore("usem") as usem,
        nc.semaphore("vsem") as vsem,
        nc.semaphore("msem") as msem,
        ExitStack() as stack,
    ):
        gsems = [stack.enter_context(nc.semaphore(f"g{j}")) for j in range(GG)]  # noqa: ANT232

        @block.sync
        def _(sync):
            sync.dma_start(out=idx[:], in_=loc[:]).then_inc(io, 16)
            # store results once the vector engine finishes all chunks
            sync.wait_ge(vsem, GG)
            sync.dma_start(out=out[:], in_=res[:]).then_inc(io, 16)
            # no explicit wait on the store sem: the end-of-block drain
            # quiesces outstanding DMA queues before the NEFF completes
            sync.wait_ge(io, 16)

        @block.gpsimd
        def _(gpsimd):
            # dma_gather lives in the loadable mlp Q7 library; the ~6us IRAM
            # fetch happens lazily at the FIRST extended instruction, so issue
            # a tiny warm-up gather before waiting on the idx load. Its idx
            # values are uninitialized SBUF (safe: any int16 stays within
            # window 0 of tab; trailing negatives just trim the count) and it
            # writes garbage rows to a scratch tile nobody reads.
            gpsimd.load_library(mlp)
            gpsimd.dma_gather(
                out_ap=warm[:],
                in_ap=tab[0:WIN],
                idxs_ap=idx[:, 0:1],
                num_idxs=16,
                num_idxs_reg=16,
                elem_size=K,
                single_packet=False,
            ).then_inc(wsem, 16)
            gpsimd.wait_ge(io, 16)
            for u in range(UG):
                gpsimd.dma_gather(
                    out_ap=gatU[:, GG * u:GG * (u + 1)],
                    in_ap=tab[u * WIN:(u + 1) * WIN],
                    idxs_ap=idx[:, UCOLS * u:UCOLS * (u + 1)],
                    num_idxs=NU,
                    num_idxs_reg=NU,
                    elem_size=K,
                    single_packet=False,
                    queue_num=u % 4,
                ).then_inc(usem, 16)
            for g in range(GG):
                gpsimd.dma_gather(
                    out_ap=gatG[:, UG * g:UG * (g + 1)],
                    in_ap=tab[(UG + g) * WIN:(UG + g + 1) * WIN],
                    idxs_ap=idx[:, UG * UCOLS + GCOLS * g:UG * UCOLS + GCOLS * (g + 1)],
                    num_idxs=NG,
                    num_idxs_reg=NG,
                    elem_size=K,
                    single_packet=False,
                    queue_num=g % 4,
                ).then_inc(gsems[g], 16)

        @block.vector
        def _(vector):
            # U tile viewed [p, wu, wg, k]; chunk j uses its wg=j plane
            uview = gatU[:].rearrange("p (a b) k -> p a b k", b=GG)
            vector.wait_ge(usem, 16 * UG)
            for j in range(GG):
                vector.wait_ge(gsems[j], 16)
                vector.tensor_mul(
                    out=gatG[:, UG * j:UG * (j + 1)],
                    in0=gatG[:, UG * j:UG * (j + 1)],
                    in1=uview[:, :, j],
                ).then_inc(msem, 1)
                # same-engine RAW still needs a sem: DVE writes drain async
                vector.wait_ge(msem, j + 1)
                vector.tensor_reduce(
                    out=res[:, UG * j:UG * (j + 1)],
                    in_=gatG[:, UG * j:UG * (j + 1)],
                    axis=mybir.AxisListType.X,
                    op=mybir.AluOpType.add,
                ).then_inc(vsem, 1)

    nc.compile()
    return nc


def _get_nc():
    if "nc" not in _CACHE:
        _CACHE["nc"] = build_nc()
    return _CACHE["nc"]


def _wrap16(seg):
    """Pack a flat idx list into the [16, n/16] wrap (idx i at [i%16, i//16])."""
    return seg.reshape(-1, 16).T


def make_in_maps(user_matrix, goods_matrix, location):
    """Host-side: window-sharded tables, per-core cell-bucketed int16 idxs."""
    user = np.ascontiguousarray(np.asarray(user_matrix), dtype=np.float32)
    goodsT = np.ascontiguousarray(np.asarray(goods_matrix).T).astype(
        np.float32, copy=False
    )
    userP = np.zeros((NWIN * WIN, K), np.float32)
    userP[:USER_NUM] = user
    goodsP = np.zeros((NWIN * WIN, K), np.float32)
    goodsP[:GOODS_NUM] = goodsT

    loc = np.asarray(location).astype(np.int64)
    l0, l1 = loc[:, 0], loc[:, 1]
    wu, wg = l0 >> 15, l1 >> 15                  # window 0..15
    core = (wu >> 2) * 2 + (wg >> 3)             # 4x2 grid
    uc, gc = wu & 3, wg & 7                      # local window in core
    cell = uc * GG + gc                          # 0..31 (wu-major id)
    # rank within (core, cell)
    key = core * CELLS + cell
    order = np.argsort(key, kind="stable")
    ks = key[order]
    starts = np.searchsorted(ks, np.arange(N_CORES * CELLS))
    rank = np.empty(BATCH, np.int64)
    rank[order] = np.arange(BATCH) - starts[ks]
    counts = np.bincount(key, minlength=N_CORES * CELLS)
    assert counts.max() <= CAP, f"cell overflow: {counts.max()} > {CAP}"

    in_maps = []
    for c in range(N_CORES):
        i, j = c >> 1, c & 1
        tab = np.concatenate(
            [userP[4 * i * WIN:(4 * i + UG) * WIN],
             goodsP[8 * j * WIN:(8 * j + GG) * WIN]], axis=0
        )
        sel = core == c
        ucc, gcc, rr = uc[sel], gc[sel], rank[sel]
        arrU = np.zeros(UG * GG * CAP, np.int16)   # U-slot order: wu-major
        arrG = np.zeros(GG * UG * CAP, np.int16)   # G-slot order: wg-major
        arrU[ucc * (GG * CAP) + gcc * CAP + rr] = (l0[sel] & 32767).astype(np.int16)
        arrG[gcc * (UG * CAP) + ucc * CAP + rr] = (l1[sel] & 32767).astype(np.int16)
        tile16 = np.concatenate(
            [_wrap16(arrU[u * NU:(u + 1) * NU]) for u in range(UG)]
            + [_wrap16(arrG[g * NG:(g + 1) * NG]) for g in range(GG)],
            axis=1,
        )
        assert tile16.shape == (16, IDXCOLS)
        in_maps.append({"tab": tab, "loc": np.tile(tile16, (8, 1))})

    meta = {"core": core, "uc": uc, "gc": gc, "rank": rank}
    return in_maps, meta


def unshard(results, meta):
    """Map per-core res[rank, wg-major cell] back to pair order."""
    res_all = np.stack([results[c]["out"] for c in range(N_CORES)])  # [8, P, 32]
    col = meta["gc"] * UG + meta["uc"]
    return res_all[meta["core"], meta["rank"], col].reshape(BATCH, 1)


def run(in_maps, trace=False, **kwargs):
    nc = _get_nc()
    return run_bass_kernel_spmd(
        nc, in_maps, core_ids=list(range(N_CORES)), trace=trace, **kwargs
    )


def kernel(user_matrix, goods_matrix, location):
    in_maps, meta = make_in_maps(user_matrix, goods_matrix, location)
    res = run(in_maps)
    return unshard(res.results, meta).astype(np.float32)


# revision 13
# speedup vs baseline: 2.3140x; 2.3140x over previous
"""Embedding-lookup dot product kernel for 8 TRN2 NeuronCores.

out[i] = dot(user_matrix[location[i,0], :], goods_matrix[:, location[i,1]])

Mechanism discovery (HW-verified): the INDIRECT1D SWDGE instruction consumes
exactly ONE offset per partition (128 rows / ~1.1us Pool-serial each -> the
58.9us baseline needs 32 of them), while the Anthropic extended DMA_GATHER
instruction amortizes descriptor generation (~1us fixed + 0.34ns/row, 8192
rows in one instruction) but takes int16 indices, so one instruction only
addresses a 32768-row (16MB) window of the source.

Strategy (model-parallel over table windows, per the sharding hint): pad each
factor matrix to 16 windows x 32768 rows. Arrange the 8 cores in a 4x2 grid:
core (i,j) owns u-windows {4i..4i+3} (64MB) and g-windows {8j..8j+7} (128MB)
of the transposed goods matrix -- a fixed, data-independent sharding. Every
pair (wu, wg) then belongs to exactly one core, and each core's gather work
is 12 dma_gather instructions (4 u + 8 g) instead of 32 INDIRECT1Ds.

Within a core, pairs are bucketed by cell (wu_local, wg_local) (4x8=32
cells), each cell padded to 128 slots so one cell = one SBUF column of the
gathered tile (dma_gather writes slot i to partition i%128, column i//128).
The U tile is filled wu-major, the G tile wg-major; the DVE multiplies the
G tile against a stride-transposed AP view of the U tile (cells align), one
chunk per g-window as it lands, then reduces over K into res. Host maps
res[rank, cell] back to pair order and drops padding slots.
"""

from contextlib import ExitStack

import numpy as np

import concourse.bacc as bacc
import concourse.mybir as mybir
from concourse import bass
from concourse.library_config import mlp
from concourse.bass_utils import run_bass_kernel_spmd

N_CORES = 8
USER_NUM = 500000
GOODS_NUM = 500000
K = 128               # embedding dim
BATCH = 16384
P = 128               # SBUF partitions

WIN = 32768           # int16-addressable rows per dma_gather window
NWIN = 16             # windows per factor matrix (16*32768 = 524288 >= 500000)
UG, GG = 4, 8         # u-windows and g-windows per core (4x2 core grid)
CELLS = UG * GG       # 32 cells per core
CAP = 128             # pair slots per cell (one SBUF column)
NU = GG * CAP         # idxs per u-gather instruction (1024)
NG = UG * CAP         # idxs per g-gather instruction (512)
UCOLS = NU // 16      # idx tile columns per u instruction (64)
GCOLS = NG // 16      # (32)
IDXCOLS = UG * UCOLS + GG * GCOLS   # 512

_CACHE = {}


def build_nc():
    """Build + compile the per-core Bass graph (identical on all 8 cores;
    only the tensor contents differ per core)."""
    f32 = mybir.dt.float32
    i16 = mybir.dt.int16

    # Skip the constructor's all-engine barrier: every cross-engine
    # dependency in this kernel is explicitly sem-ordered, and the const-AP
    # memsets it protects are unused here.
    orig_barrier = bass.Bass.all_engine_barrier
    bass.Bass.all_engine_barrier = lambda self, *, sem_only=False: None
    try:
        nc = bacc.Bacc(
            "TRN2",
            target_bir_lowering=False,
            debug=False,
            num_devices=N_CORES,
            enable_partition_id=False,
            monotonic_sem_count=0,
            num_swdge_queues=4,
        )
    finally:
        bass.Bass.all_engine_barrier = orig_barrier

    # per-core table: this core's 4 u-windows then its 8 g-windows
    tab = nc.dram_tensor("tab", [(UG + GG) * WIN, K], f32, kind="ExternalInput")
    loc = nc.dram_tensor("loc", [P, IDXCOLS], i16, kind="ExternalInput")
    out = nc.dram_tensor("out", [P, CELLS], f32, kind="ExternalOutput")

    with (
        # all SWDGE completions are sem-proven before the store, so skip the
        # expensive gpsimd dge_drain at block end
        nc.Block(no_gpsimd_drain=True) as block,
        nc.sbuf_tensor("idx", [P, IDXCOLS], i16) as idx,
        nc.sbuf_tensor("gatU", [P, UG * GG, K], f32) as gatU,   # wu-major cells
        nc.sbuf_tensor("gatG", [P, GG * UG, K], f32) as gatG,   # wg-major cells
        nc.sbuf_tensor("res", [P, CELLS], f32) as res,
        nc.semaphore("io") as io,
        nc.semaphore("usem") as usem,
        nc.semaphore("vsem") as vsem,
        nc.semaphore("msem") as msem,
        ExitStack() as stack,
    ):
        gsems = [stack.enter_context(nc.semaphore(f"g{j}")) for j in range(GG)]  # noqa: ANT232

        @block.sync
        def _(sync):
            sync.dma_start(out=idx[:], in_=loc[:]).then_inc(io, 16)
            # store results once the vector engine finishes all chunks
            sync.wait_ge(vsem, GG)
            sync.dma_start(out=out[:], in_=res[:]).then_inc(io, 16)
            # no explicit wait on the store sem: the end-of-block drain
            # quiesces outstanding DMA queues before the NEFF completes
            sync.wait_ge(io, 16)

        @block.gpsimd
        def _(gpsimd):
            # dma_gather lives in the loadable mlp Q7 library; the ~6us IRAM
            # fetch happens lazily at the first extended instruction
            gpsimd.load_library(mlp)
            gpsimd.wait_ge(io, 16)
            for u in range(UG):
                gpsimd.dma_gather(
                    out_ap=gatU[:, GG * u:GG * (u + 1)],
                    in_ap=tab[u * WIN:(u + 1) * WIN],
                    idxs_ap=idx[:, UCOLS * u:UCOLS * (u + 1)],
                    num_idxs=NU,
                    num_idxs_reg=NU,
                    elem_size=K,
                    queue_num=u % 4,
                ).then_inc(usem, 16)
            for g in range(GG):
                gpsimd.dma_gather(
                    out_ap=gatG[:, UG * g:UG * (g + 1)],
                    in_ap=tab[(UG + g) * WIN:(UG + g + 1) * WIN],
                    idxs_ap=idx[:, UG * UCOLS + GCOLS * g:UG * UCOLS + GCOLS * (g + 1)],
                    num_idxs=NG,
                    num_idxs_reg=NG,
                    elem_size=K,
                    queue_num=g % 4,
                ).then_inc(gsems[g], 16)

        @block.vector
        def _(vector):
            # U tile viewed [p, wu, wg, k]; chunk j uses its wg=j plane
            uview = gatU[:].rearrange("p (a b) k -> p a b k", b=GG)
            vector.wait_ge(usem, 16 * UG)
            for j in range(GG):
                vector.wait_ge(gsems[j], 16)
                vector.tensor_mul(
                    out=gatG[:, UG * j:UG * (j + 1)],
                    in0=gatG[:, UG * j:UG * (j + 1)],
                    in1=uview[:, :, j],
                ).then_inc(msem, 1)
                # same-engine RAW still needs a sem: DVE writes drain async
                vector.wait_ge(msem, j + 1)
                vector.tensor_reduce(
                    out=res[:, UG * j:UG * (j + 1)],
                    in_=gatG[:, UG * j:UG * (j + 1)],
                    axis=mybir.AxisListType.X,
                    op=mybir.AluOpType.add,
                ).then_inc(vsem, 1)

    nc.compile()
    return nc


def _get_nc():
    if "nc" not in _CACHE:
        _CACHE["nc"] = build_nc()
    return _CACHE["nc"]


def _wrap16(seg):
    """Pack a flat idx list into the [16, n/16] wrap (idx i at [i%16, i//16])."""
    return seg.reshape(-1, 16).T


def make_in_maps(user_matrix, goods_matrix, location):
    """Host-side: window-sharded tables, per-core cell-bucketed int16 idxs."""
    user = np.ascontiguousarray(np.asarray(user_matrix), dtype=np.float32)
    goodsT = np.ascontiguousarray(np.asarray(goods_matrix).T).astype(
        np.float32, copy=False
    )
    userP = np.zeros((NWIN * WIN, K), np.float32)
    userP[:USER_NUM] = user
    goodsP = np.zeros((NWIN * WIN, K), np.float32)
    goodsP[:GOODS_NUM] = goodsT

    loc = np.asarray(location).astype(np.int64)
    l0, l1 = loc[:, 0], loc[:, 1]
    wu, wg = l0 >> 15, l1 >> 15                  # window 0..15
    core = (wu >> 2) * 2 + (wg >> 3)             # 4x2 grid
    uc, gc = wu & 3, wg & 7                      # local window in core
    cell = uc * GG + gc                          # 0..31 (wu-major id)
    # rank within (core, cell)
    key = core * CELLS + cell
    order = np.argsort(key, kind="stable")
    ks = key[order]
    starts = np.searchsorted(ks, np.arange(N_CORES * CELLS))
    rank = np.empty(BATCH, np.int64)
    rank[order] = np.arange(BATCH) - starts[ks]
    counts = np.bincount(key, minlength=N_CORES * CELLS)
    assert counts.max() <= CAP, f"cell overflow: {counts.max()} > {CAP}"

    in_maps = []
    for c in range(N_CORES):
        i, j = c >> 1, c & 1
        tab = np.concatenate(
            [userP[4 * i * WIN:(4 * i + UG) * WIN],
             goodsP[8 * j * WIN:(8 * j + GG) * WIN]], axis=0
        )
        sel = core == c
        ucc, gcc, rr = uc[sel], gc[sel], rank[sel]
        # padding slots get indices spread across the window (NOT all row 0:
        # thousands of descriptors hitting one 512B line serialize on a
        # single DRAM bank)
        arrU = ((np.arange(UG * GG * CAP) * 5237) % WIN).astype(np.int16)
        arrG = ((np.arange(GG * UG * CAP) * 5237) % WIN).astype(np.int16)
        arrU[ucc * (GG * CAP) + gcc * CAP + rr] = (l0[sel] & 32767).astype(np.int16)
        arrG[gcc * (UG * CAP) + ucc * CAP + rr] = (l1[sel] & 32767).astype(np.int16)
        tile16 = np.concatenate(
            [_wrap16(arrU[u * NU:(u + 1) * NU]) for u in range(UG)]
            + [_wrap16(arrG[g * NG:(g + 1) * NG]) for g in range(GG)],
            axis=1,
        )
        assert tile16.shape == (16, IDXCOLS)
        in_maps.append({"tab": tab, "loc": np.tile(tile16, (8, 1))})

    meta = {"core": core, "uc": uc, "gc": gc, "rank": rank}
    return in_maps, meta


def unshard(results, meta):
    """Map per-core res[rank, wg-major cell] back to pair order."""
    res_all = np.stack([results[c]["out"] for c in range(N_CORES)])  # [8, P, 32]
    col = meta["gc"] * UG + meta["uc"]
    return res_all[meta["core"], meta["rank"], col].reshape(BATCH, 1)


def run(in_maps, trace=False, **kwargs):
    nc = _get_nc()
    return run_bass_kernel_spmd(
        nc, in_maps, core_ids=list(range(N_CORES)), trace=trace, **kwargs
    )


def kernel(user_matrix, goods_matrix, location):
    in_maps, meta = make_in_maps(user_matrix, goods_matrix, location)
    res = run(in_maps)
    return unshard(res.results, meta).astype(np.float32)
